# revision 16
# baseline (speedup 1.0000x reference)
"""Trainium2 Bass kernel for nn_Canvas_by_Distance (vq_codebook).

Math: the reference's StraightThroughSoftMax forward is numerically
hard one-hot(argmax of squared distances), so the output is
    out = nearest_upsample_4x( palette[argmax_c ||sigmoid(w) - p_c||^2] )

Key input-adaptive optimization (computed on host, baked at build):
sigmoid(weight) lives in a tight per-channel box [lo, hi].  For two
colors, dist_{c'}(w) - dist_c(w) is LINEAR in w, so "c' always beats c
on the box" is certified by checking the 8 box corners.  Colors that
are dominated can never be the argmax and are pruned; typically only
K ~ 3 of 16 survive, collapsing the per-pixel work.

Device algorithm per core (canvas rows sharded 8-ways, 128 rows/core),
pipelined over column chunks (small chunks first so the output DMA
stream starts early):
  - one DMA per chunk loads all 3 channels; one ACT op applies sigmoid
  - per surviving color (j = 0..K-1, ascending original index), fused
    custom-DVE ops (same fp32 add order as the jax reference, then
    scaled + clamped and rounded to i32):
        u   = (w0-p0)^2 + (w1-p1)^2                      (CBD_SQD2)
        s_q = i32(min(((w2-p2)^2 + u) * QSCALE, CLAMP))  (CBD_SQDA)
  - packed int32 tournament on DVE:
        cand   = (s_q << 4) | (15 - j)
        packed = max(packed, cand)   # on f32-BITCAST views: positive
                                     # IEEE order == int32 order, so the
                                     # argmax is exact to fp32 precision;
                                     # payload 15-j makes ties pick the
                                     # smallest index (matches jnp.argmax)
  - v = packed & 15 (= 15 - j)
  - palette lookup + x4 column replication in one pass: custom-DVE
    2-values-per-op selects (CBD_MAP2) read v through a step-0
    broadcast AP at full output width and write bf16 directly
  - 4x row replication inside one output DMA per channel per chunk
    (step-0 SBUF read AP), alternating the two HWDGE rings

The output DRAM tensor is bf16 (the palette colors survive bf16
rounding at ~1e-3 relative error, far inside the 2e-2 gate); the host
gather upcasts to fp32.  This halves the dominant HBM write traffic.

Palette values / pruning are baked into the instruction stream as
immediates (the kernel is rebuilt per call; inputs are runtime data to
the harness but compile-time constants to the NEFF).
"""

import math
import os

import numpy as np

CH, CW = 1024, 1024          # canvas
OH, OW = 4096, 4096          # image
NCOLORS = 16
NCORES = 8
RPC = CH // NCORES           # canvas rows per core = 128
ORPC = RPC * 4               # output rows per core = 512
# packed = s_q*16 + 15 must stay <= 0x7F7FFFFF so the f32 bitcast is a
# finite positive float; clamp s_q accordingly.
QCLAMP = 133693432.0

# column chunking of the 1024 canvas columns (pipeline compute vs DMA-out);
# LGROUPS batches consecutive chunks into one input DMA (SWDGE gen on the
# Pool engine costs ~1.1us per load, so per-chunk loads pace arrivals too
# slowly during warmup)
CHUNKS = tuple(
    int(x) for x in os.environ.get(
        "CBD_CHUNKS", "64,96,128,160,192,192,192"
    ).split(",")
)
assert sum(CHUNKS) == CW
LGROUPS = tuple(
    int(x) for x in os.environ.get("CBD_LG", "1,2,2,2").split(",")
)
assert sum(LGROUPS) == len(CHUNKS)

_OPS_CACHE = {}
_MODULE_CACHE = {}


def _register_ops():
    """Register the custom DVE ops (idempotent)."""
    if _OPS_CACHE:
        return _OPS_CACHE

    import concourse.dve_ops as dve_ops
    from concourse.dve_spec import (
        C0, C1, C2, One, Spec, Src0, Src1, _has_src1, eq, lower, minn,
        select, sq,
    )
    from concourse.dve_uop import DveOpSpec

    f32 = np.float32

    def register(name, spec, subdim=False):
        if name in dve_ops._SUB_OPCODE_FOR_NAME:
            return next(o for o in dve_ops.OPS if o.name == name)
        row = dve_ops._CUSTOM_DVE_ROW_BASE + len(dve_ops.OPS)
        assert row < 0x20, "custom DVE opcode rows exhausted"
        dve_ops._SUB_OPCODE_FOR_NAME[name] = row
        shas = {}
        for ver in ("v3", "v4"):
            uops = lower(spec, ver=ver)
            shas[ver] = DveOpSpec(
                name=name, opcode=row, uops=uops, rd1_en=_has_src1(spec)
            ).sha(ver)
        op = dve_ops.DveOp(name, spec, subdim=subdim, uops_sha=shas)
        dve_ops.OPS.append(op)
        dve_ops.CUSTOM_DVE_SPECS[name] = spec
        return op

    _OPS_CACHE["SQD2"] = register(
        "CBD_SQD2",
        Spec(
            body=sq(Src0 - C0) + sq(Src1 - C1),
            reference=lambda in0, in1, s0, s1, imm2: np.square(in0 - f32(s0))
            + np.square(in1 - f32(s1)),
        ),
    )
    _OPS_CACHE["SQDA"] = register(
        "CBD_SQDA",
        Spec(
            body=minn((sq(Src0 - C0) + Src1) * C1, C2),
            reference=lambda in0, in1, s0, s1, imm2: np.minimum(
                (np.square(in0 - f32(s0)) + in1) * f32(s1), f32(imm2)
            ),
        ),
    )
    def _map2_ref(in0, in1, s0, s1, imm2):
        in0 = np.asarray(in0, np.float32)
        in1 = np.asarray(in1, np.float32)
        if in1.shape != in0.shape:
            if in1.size == in0.size:  # same elements, different AP shape
                in1 = in1.reshape(in0.shape)
            else:  # [P,1] broadcast Src1
                in1 = in1.reshape(in1.shape[0], *([1] * (in0.ndim - 1)))
        return np.where(
            in0 == f32(s1),
            f32(s0),
            np.where(in0 - f32(1.0) == f32(s1), f32(imm2), in1),
        ).astype(np.float32)

    _OPS_CACHE["MAP2"] = register(
        "CBD_MAP2",
        Spec(
            body=select(eq(Src0, C1), C0, select(eq(Src0 - One, C1), C2, Src1)),
            reference=_map2_ref,
        ),
    )
    return _OPS_CACHE


def _prune_palette(weight, pal):
    """Survivor color indices (ascending) + score upper bound over the box.

    A color c is pruned when some c' strictly dominates it on the whole
    sigmoid(weight) box: dist_{c'} - dist_c is linear in w, so checking
    the 8 corners suffices.  Margins cover host-vs-device sigmoid error.
    """
    wmin = weight.min(axis=(1, 2)).astype(np.float64)
    wmax = weight.max(axis=(1, 2)).astype(np.float64)
    lo = np.clip(1.0 / (1.0 + np.exp(-wmin)) - 1e-4, 0.0, 1.0)
    hi = np.clip(1.0 / (1.0 + np.exp(-wmax)) + 1e-4, 0.0, 1.0)
    corners = np.array(
        [[(lo, hi)[(i >> d) & 1][d] for d in range(3)] for i in range(8)]
    )
    p = pal.astype(np.float64)
    pnorm = (p * p).sum(axis=1)
    dominated = np.zeros(NCOLORS, dtype=bool)
    for c in range(NCOLORS):
        for cp in range(NCOLORS):
            if cp == c:
                continue
            g = -2.0 * corners @ (p[cp] - p[c]) + (pnorm[cp] - pnorm[c])
            if g.min() > 1e-3:
                dominated[c] = True
                break
    surv = [c for c in range(NCOLORS) if not dominated[c]]
    # max possible score over the box (extreme at a corner per color)
    s_ub = float(((corners[:, None, :] - p[None, :, :]) ** 2).sum(-1).max()) * 1.05
    return surv, s_ub


def _body(tc, nc, out_t, w_t, pal, surv, qscale, iters=1):
    """Emit the per-core program; palette/pruning baked as immediates."""
    from contextlib import ExitStack

    import concourse.mybir as mybir

    ops = _register_ops()

    f32 = mybir.dt.float32
    bf16 = mybir.dt.bfloat16
    Act = mybir.ActivationFunctionType

    K = len(surv)
    w_ap = w_t.ap()                                            # (3, 128, 1024)
    out_r = out_t.ap().rearrange("c (p k) w -> c p k w", k=4)  # (3,128,4,4096)

    ctx = ExitStack()
    p_w = ctx.enter_context(tc.tile_pool(name="w", bufs=4))
    p_sg = ctx.enter_context(tc.tile_pool(name="sg", bufs=4))
    p_tmp = ctx.enter_context(tc.tile_pool(name="tmp", bufs=4))
    p_map = ctx.enter_context(tc.tile_pool(name="map", bufs=2))
    p_rep = ctx.enter_context(tc.tile_pool(name="rep", bufs=3))
    p_const = ctx.enter_context(tc.tile_pool(name="const", bufs=1))

    # persistent full-width fallback constant tiles for the 1-op map path
    # (a [P,1]-broadcast Src1 fails on HW; a full 2-D tensor works).  Only
    # the chunk-0/1 prefix is memset up front — the rest is filled while
    # the DVE would otherwise idle after the first chunks — so the first
    # output DMA isn't delayed behind 3 full-width memsets.
    fbw = []
    wmax = 4 * max(CHUNKS)
    w0 = 4 * max(CHUNKS[: min(2, len(CHUNKS))])
    if 2 <= K <= 3:
        for d in range(3):
            t = p_const.tile([RPC, wmax], bf16, tag=f"fbw{d}")
            nc.vector.memset(t[:, :w0], float(pal[surv[-1], d]))
            fbw.append(t)

    for _ in range(iters):
        ci = 0
        col0 = 0
        for gi, ng in enumerate(LGROUPS):
            gchunks = CHUNKS[ci : ci + ng]
            Fg = sum(gchunks)
            sgt = None
            if K > 1:
                # one DMA + one sigmoid for all 3 channels of this group.
                # Group 0 loads via the SP HWDGE ring: it skips the Pool
                # engine's startup memsets and its ~1.1us SWDGE descriptor
                # generation, shaving the critical path to the first output.
                wt = p_w.tile([RPC, 3 * Fg], f32, tag="w")
                eng = nc.sync if gi == 0 else nc.gpsimd
                eng.dma_start(
                    wt[:].rearrange("p (c f) -> p c f", c=3),
                    w_ap[:, :, col0 : col0 + Fg].rearrange("c p f -> p c f"),
                )
            off = 0
            for F in gchunks:
                sg = None
                if K > 1:
                    # per-chunk sigmoid: the first chunk's DVE work starts
                    # as soon as its own columns are activated
                    sgt = p_sg.tile([RPC, 3 * F], f32, tag="sg")
                    wt_v = wt[:].rearrange("p (c f) -> p c f", c=3)
                    nc.scalar.activation(
                        sgt[:].rearrange("p (c f) -> p c f", c=3),
                        wt_v[:, :, off : off + F], Act.Sigmoid,
                    )
                    sg = [sgt[:, d * F : (d + 1) * F] for d in range(3)]
                _chunk(tc, nc, out_r, pal, surv, qscale, col0 + off, F, ci,
                       sg, fbw, p_tmp, p_map, p_rep, p_w, ops, mybir)
                if ci == 1 and fbw and wmax > w0:
                    for d in range(3):
                        nc.vector.memset(
                            fbw[d][:, w0:], float(pal[surv[-1], d])
                        )
                off += F
                ci += 1
            col0 += Fg

    ctx.close()


def _chunk(tc, nc, out_r, pal, surv, qscale, col0, F, ci, sg, fbw,
           p_tmp, p_map, p_rep, p_w, ops, mybir):
    SQD2, SQDA, MAP2 = ops["SQD2"], ops["SQDA"], ops["MAP2"]
    f32 = mybir.dt.float32
    bf16 = mybir.dt.bfloat16
    i32 = mybir.dt.int32
    Alu = mybir.AluOpType
    K = len(surv)

    if K == 1:
        # single possible winner: output is a constant color
        for d in range(3):
            rep = p_rep.tile([RPC, 4 * F], bf16, tag=f"rep{d}")
            nc.vector.memset(rep[:], float(pal[surv[0], d]))
            rep_b = rep[:].unsqueeze(1).broadcast_to([RPC, 4, 4 * F])
            nc.sync.dma_start(
                out_r[d, :, :, 4 * col0 : 4 * col0 + 4 * F], rep_b
            )
        return

    # --- surviving-color scores + packed int32 tournament -------------------
    packed = p_w.tile([RPC, F], i32, tag="packed")
    for j, c in enumerate(surv):
        u = p_tmp.tile([RPC, F], f32, tag="u")
        nc.vector._custom_dve(
            SQD2, out=u[:], in0=sg[0], in1=sg[1],
            s0=float(pal[c, 0]), s1=float(pal[c, 1]),
        )
        sq_ = p_tmp.tile([RPC, F], i32, tag="sq")
        nc.vector._custom_dve(
            SQDA, out=sq_[:], in0=sg[2], in1=u[:],
            s0=float(pal[c, 2]), s1=qscale, imm2=QCLAMP,
        )
        if j == 0:
            nc.vector.tensor_scalar(
                packed[:], sq_[:], 4, 15 - j,
                Alu.arith_shift_left, Alu.bitwise_or,
            )
        else:
            cand = p_tmp.tile([RPC, F], i32, tag="cand")
            nc.vector.tensor_scalar(
                cand[:], sq_[:], 4, 15 - j,
                Alu.arith_shift_left, Alu.bitwise_or,
            )
            # on f32-BITCAST views: positive IEEE order == int32 order, so
            # the argmax is exact to fp32 precision; payload 15-j makes
            # ties pick the smallest index (matches jnp.argmax)
            nc.vector.tensor_max(
                packed[:].bitcast(f32), packed[:].bitcast(f32),
                cand[:].bitcast(f32),
            )

    # v = packed & 15  (= 15 - j), cast to f32 by the output converter
    idx = p_w.tile([RPC, F], f32, tag="idx")
    nc.vector.tensor_scalar(idx[:], packed[:], 15, None, Alu.bitwise_and)
    # x4 column replication comes free: MAP2 reads idx through a step-0
    # broadcast AP at full output width and writes the bf16 staging tile
    idx_b = idx[:].unsqueeze(2).broadcast_to([RPC, F, 4])

    # --- palette map + x4 column replication, store --------------------------
    # v-value 15-j  <->  color surv[j];  v ranges over [16-K, 15]
    v2c = {15 - j: c for j, c in enumerate(surv)}
    vlo = 16 - K - (K % 2)
    vpairs = list(range(vlo, 16, 2))
    use_wide = K <= 4
    for d in range(3):
        rep = p_rep.tile([RPC, 4 * F], bf16, tag=f"rep{d}")
        if use_wide and K <= 3 and fbw:
            # single op: {14 -> s0, 15 -> imm2, else -> fallback tile}
            nc.vector._custom_dve(
                MAP2, out=rep[:], in0=idx_b, in1=fbw[d][:, : 4 * F],
                s0=float(pal[v2c[14], d]) if 14 in v2c
                else float(pal[surv[0], d]),
                s1=14.0,
                imm2=float(pal[v2c[15], d]),
            )
        elif use_wide:
            cur = None
            for v in vpairs:
                last = v == vpairs[-1]
                nxt_dt = bf16 if last else f32
                nxt = rep if last else p_map.tile(
                    [RPC, 4 * F], f32, tag=f"mw{d}"
                )
                nc.vector._custom_dve(
                    MAP2, out=nxt[:], in0=idx_b,
                    in1=(cur[:] if cur is not None else fbw_fill(
                        tc, nc, p_map, d, F, pal, surv, mybir)),
                    s0=float(pal[v2c.get(v, surv[-1]), d]), s1=float(v),
                    imm2=float(pal[v2c.get(v + 1, surv[-1]), d]),
                )
                cur = nxt
        else:
            # general chain at width F, then replicate via ACT copies
            cur = None
            for v in vpairs:
                nxt = p_map.tile([RPC, F], f32, tag=f"m{d}")
                nc.vector._custom_dve(
                    MAP2, out=nxt[:], in0=idx[:],
                    in1=(cur[:] if cur is not None else fbw_fill(
                        tc, nc, p_map, d, F, pal, surv, mybir, wide=False)),
                    s0=float(pal[v2c.get(v, surv[-1]), d]), s1=float(v),
                    imm2=float(pal[v2c.get(v + 1, surv[-1]), d]),
                )
                cur = nxt
            rep_v = rep[:].rearrange("p (j k) -> p j k", k=4)
            for k in range(4):
                nc.scalar.copy(rep_v[:, :, k], cur[:])
        # one DMA per channel: write 4 replicated rows from a step-0
        # read of the same SBUF data.  All outs go on the SP ring by
        # default: the ACT queue then carries only sigmoids, so a
        # pending out-DMA's sem wait never delays the next chunk's
        # sigmoid dispatch (ACT ring mode: alternate rings per channel).
        if os.environ.get("CBD_RING", "sp") == "sp":
            eng = nc.sync
        else:
            eng = nc.sync if ((d + ci) % 2 == 0) else nc.scalar
        rep_b = rep[:].unsqueeze(1).broadcast_to([RPC, 4, 4 * F])
        eng.dma_start(out_r[d, :, :, 4 * col0 : 4 * col0 + 4 * F], rep_b)


def fbw_fill(tc, nc, p_map, d, F, pal, surv, mybir, wide=True):
    """Fallback-value seed tile for MAP2 chains without persistent fbw."""
    w = 4 * F if wide else F
    t = p_map.tile([RPC, w], mybir.dt.float32, tag=f"seed{d}")
    nc.vector.memset(t[:], float(pal[surv[-1], d]))
    return t[:]


def build_module(weight, pal):
    """Build + compile the single-core Bass program (palette baked in)."""
    surv, s_ub = _prune_palette(weight, pal)
    qscale = float(2.0 ** min(30, int(math.floor(math.log2(QCLAMP / s_ub)))))
    iters = int(os.environ.get("CBD_ITERS", "1"))
    key = (pal.astype(np.float32).tobytes(), tuple(surv), qscale, iters, CHUNKS)
    if key in _MODULE_CACHE:
        return _MODULE_CACHE[key]

    import concourse.bacc as bacc
    import concourse.mybir as mybir
    import concourse.tile as tile

    nc = bacc.Bacc("TRN2", target_bir_lowering=False, debug=False)
    w_in = nc.dram_tensor("w", [3, RPC, CW], mybir.dt.float32, kind="ExternalInput")
    out = nc.dram_tensor(
        "out", [3, ORPC, OW], mybir.dt.bfloat16, kind="ExternalOutput"
    )
    with tile.TileContext(nc) as tc:
        _body(tc, nc, out, w_in, pal, surv, qscale, iters=iters)
    nc.compile()
    _MODULE_CACHE[key] = nc
    return nc


def kernel(weight, palette):
    """Full inputs in, full output out. Shards rows across 8 NeuronCores."""
    from concourse.bass_utils import run_bass_kernel_spmd

    weight = np.ascontiguousarray(weight, dtype=np.float32)
    pal = np.ascontiguousarray(palette, dtype=np.float32)
    assert weight.shape == (3, CH, CW) and pal.shape == (NCOLORS, 3)

    nc = build_module(weight, pal)

    in_maps = [
        {"w": np.ascontiguousarray(weight[:, m * RPC : (m + 1) * RPC, :])}
        for m in range(NCORES)
    ]
    trace = bool(int(os.environ.get("CBD_TRACE", "0")))
    res = run_bass_kernel_spmd(
        nc, in_maps, core_ids=list(range(NCORES)), trace=trace
    )
    kernel.last_results = res

    full = np.empty((3, OH, OW), dtype=np.float32)
    for m in range(NCORES):
        full[:, m * ORPC : (m + 1) * ORPC, :] = np.asarray(
            res.results[m]["out"]
        ).astype(np.float32)
    return full


# revision 17
# speedup vs baseline: 1.4608x; 1.4608x over previous
"""Trainium2 Bass kernel for nn_Canvas_by_Distance (vq_codebook).

Math: the reference's StraightThroughSoftMax forward is numerically
hard one-hot(argmax of squared distances), so the output is
    out = nearest_upsample_4x( palette[argmax_c ||sigmoid(w) - p_c||^2] )

Key input-adaptive optimization (computed on host, baked at build):
sigmoid(weight) lives in a tight per-channel box [lo, hi].  For two
colors, dist_{c'}(w) - dist_c(w) is LINEAR in w, so "c' always beats c
on the box" is certified by checking the 8 box corners.  Colors that
are dominated can never be the argmax and are pruned; typically only
K ~ 3 of 16 survive, collapsing the per-pixel work.

The output is written as uint8 (round(color*255); the host gather
decodes /255).  The palette colors survive 8-bit quantization at
~7e-3 global relative error, far inside the 2e-2 gate, and the
dominant HBM write traffic drops 4x vs fp32.

Device algorithm per core (canvas rows sharded 8-ways, 128 rows/core),
pipelined over column chunks:
  - one DMA per load-group of chunks (all 3 channels), sigmoid per
    chunk on ACT
  - per surviving color (j ascending), fused custom-DVE ops:
        u   = (w0-p0)^2 + (w1-p1)^2                      (CBD_SQD2)
        s_q = i32(min(((w2-p2)^2 + u) * QSCALE, CLAMP))  (CBD_SQDA)
        pk  = i32(max(s_q*B + (B-1-j), pk_prev))         (CBD_PKMX)
    For K <= 4 the payload is 2 bits (B=4) and QSCALE caps s_q < 2^22,
    so every CBD_PKMX intermediate (< 2^24) is exact in fp32; payload
    B-1-j makes ties pick the smallest index (matches jnp.argmax).
    For K >= 5 the payload is 4 bits and the tournament falls back to
    tensor_scalar pack + f32-bitcast tensor_max (positive IEEE order
    == int32 order), exact to fp32 precision.
  - v = pk & (B-1), cast f32 by the output converter
  - palette map at CANVAS width: chained custom-DVE 2-values-per-op
    selects (CBD_MAP2) write u16 values byte*257 — i.e. two identical
    u8 bytes, which is the first 2x of the 4x column upsample for free
  - ACT pair-copy (step-0 read AP) doubles u16 elements: the second 2x
  - one output DMA per channel per chunk writes the u16 tile through a
    narrowing u8 bitcast, replicating 4 rows via a step-0 read AP

Palette values / pruning are baked into the instruction stream as
immediates (the kernel is rebuilt per call; inputs are runtime data to
the harness but compile-time constants to the NEFF).
"""

import math
import os

import numpy as np

CH, CW = 1024, 1024          # canvas
OH, OW = 4096, 4096          # image
NCOLORS = 16
NCORES = 8
RPC = CH // NCORES           # canvas rows per core = 128
ORPC = RPC * 4               # output rows per core = 512
# K<=4 tournament: s_q < 2^22 keeps pk = s_q*4 + payload < 2^24 exact in
# fp32 through the CBD_PKMX mul/add/max chain.
QCLAMP22 = float((1 << 22) - 1)
# K>=5 fallback: packed = s_q*16 + 15 must stay <= 0x7F7FFFFF for the
# f32-bitcast max trick.
QCLAMP27 = 133693432.0

# column chunking of the 1024 canvas columns (pipeline compute vs DMA-out);
# LGROUPS batches consecutive chunks into one input DMA (SWDGE gen on the
# Pool engine costs ~1.1us per load, so per-chunk loads pace arrivals too
# slowly during warmup)
CHUNKS = tuple(
    int(x) for x in os.environ.get(
        "CBD_CHUNKS", "64,96,112,144,176,208,224"
    ).split(",")
)
assert sum(CHUNKS) == CW
LGROUPS = tuple(
    int(x) for x in os.environ.get("CBD_LG", "1,1,2,3").split(",")
)
assert sum(LGROUPS) == len(CHUNKS)

_OPS_CACHE = {}
_MODULE_CACHE = {}


def _register_ops():
    """Register the custom DVE ops (idempotent)."""
    if _OPS_CACHE:
        return _OPS_CACHE

    import concourse.dve_ops as dve_ops
    from concourse.dve_spec import (
        C0, C1, C2, One, Spec, Src0, Src1, _has_src1, eq, lower, maxx, minn,
        select, sq,
    )
    from concourse.dve_uop import DveOpSpec

    f32 = np.float32

    def register(name, spec, subdim=False):
        if name in dve_ops._SUB_OPCODE_FOR_NAME:
            return next(o for o in dve_ops.OPS if o.name == name)
        row = dve_ops._CUSTOM_DVE_ROW_BASE + len(dve_ops.OPS)
        assert row < 0x20, "custom DVE opcode rows exhausted"
        dve_ops._SUB_OPCODE_FOR_NAME[name] = row
        shas = {}
        for ver in ("v3", "v4"):
            uops = lower(spec, ver=ver)
            shas[ver] = DveOpSpec(
                name=name, opcode=row, uops=uops, rd1_en=_has_src1(spec)
            ).sha(ver)
        op = dve_ops.DveOp(name, spec, subdim=subdim, uops_sha=shas)
        dve_ops.OPS.append(op)
        dve_ops.CUSTOM_DVE_SPECS[name] = spec
        return op

    _OPS_CACHE["SQD2"] = register(
        "CBD_SQD2",
        Spec(
            body=sq(Src0 - C0) + sq(Src1 - C1),
            reference=lambda in0, in1, s0, s1, imm2: np.square(in0 - f32(s0))
            + np.square(in1 - f32(s1)),
        ),
    )
    _OPS_CACHE["SQDA"] = register(
        "CBD_SQDA",
        Spec(
            body=minn((sq(Src0 - C0) + Src1) * C1, C2),
            reference=lambda in0, in1, s0, s1, imm2: np.minimum(
                (np.square(in0 - f32(s0)) + in1) * f32(s1), f32(imm2)
            ),
        ),
    )
    # pk = max(in0*C1 + payload, in1); caller keeps all values < 2^24 so
    # the fp32 mul/add/max chain is exact
    _OPS_CACHE["PKMX"] = register(
        "CBD_PKMX",
        Spec(
            body=maxx(Src0 * C1 + C0, Src1),
            reference=lambda in0, in1, s0, s1, imm2: np.maximum(
                np.asarray(in0, f32) * f32(s1) + f32(s0),
                np.asarray(in1, f32),
            ),
        ),
    )

    def _map2_ref(in0, in1, s0, s1, imm2):
        in0 = np.asarray(in0, np.float32)
        in1 = np.asarray(in1, np.float32)
        if in1.shape != in0.shape:
            if in1.size == in0.size:  # same elements, different AP shape
                in1 = in1.reshape(in0.shape)
            else:  # [P,1] broadcast Src1
                in1 = in1.reshape(in1.shape[0], *([1] * (in0.ndim - 1)))
        return np.where(
            in0 == f32(s1),
            f32(s0),
            np.where(in0 - f32(1.0) == f32(s1), f32(imm2), in1),
        ).astype(np.float32)

    _OPS_CACHE["MAP2"] = register(
        "CBD_MAP2",
        Spec(
            body=select(eq(Src0, C1), C0, select(eq(Src0 - One, C1), C2, Src1)),
            reference=_map2_ref,
        ),
    )
    return _OPS_CACHE


def _prune_palette(weight, pal):
    """Survivor color indices (ascending) + score upper bound over the box.

    A color c is pruned when some c' strictly dominates it on the whole
    sigmoid(weight) box: dist_{c'} - dist_c is linear in w, so checking
    the 8 corners suffices.  Margins cover host-vs-device sigmoid error.
    """
    wmin = weight.min(axis=(1, 2)).astype(np.float64)
    wmax = weight.max(axis=(1, 2)).astype(np.float64)
    lo = np.clip(1.0 / (1.0 + np.exp(-wmin)) - 1e-4, 0.0, 1.0)
    hi = np.clip(1.0 / (1.0 + np.exp(-wmax)) + 1e-4, 0.0, 1.0)
    corners = np.array(
        [[(lo, hi)[(i >> d) & 1][d] for d in range(3)] for i in range(8)]
    )
    p = pal.astype(np.float64)
    pnorm = (p * p).sum(axis=1)
    dominated = np.zeros(NCOLORS, dtype=bool)
    for c in range(NCOLORS):
        for cp in range(NCOLORS):
            if cp == c:
                continue
            g = -2.0 * corners @ (p[cp] - p[c]) + (pnorm[cp] - pnorm[c])
            if g.min() > 1e-3:
                dominated[c] = True
                break
    surv = [c for c in range(NCOLORS) if not dominated[c]]
    # max possible score over the box (extreme at a corner per color)
    s_ub = float(((corners[:, None, :] - p[None, :, :]) ** 2).sum(-1).max()) * 1.05
    return surv, s_ub


def _b257(pal, c, d):
    """uint16 value of color (c, d): the u8 byte replicated into both
    byte lanes (b*257), so a u16 element IS two upsampled u8 pixels."""
    return float(int(round(float(pal[c, d]) * 255.0)) * 257)


def _body(tc, nc, out_t, w_t, pal, surv, qscale, iters=1):
    """Emit the per-core program; palette/pruning baked as immediates."""
    from contextlib import ExitStack

    import concourse.mybir as mybir

    ops = _register_ops()
    SQD2, SQDA, PKMX, MAP2 = ops["SQD2"], ops["SQDA"], ops["PKMX"], ops["MAP2"]

    f32 = mybir.dt.float32
    i32 = mybir.dt.int32
    u16 = mybir.dt.uint16
    u8 = mybir.dt.uint8
    Act = mybir.ActivationFunctionType
    Alu = mybir.AluOpType

    K = len(surv)
    n = len(CHUNKS)
    w_ap = w_t.ap()                                            # (3, 128, 1024)
    out_r = out_t.ap().rearrange("c (p k) w -> c p k w", k=4)  # (3,128,4,4096)

    # payload width: 2 bits for K<=4 (enables the exact-fp32 fused PKMX
    # tournament), 4 bits + bitcast-max fallback for K>=5
    fused = 2 <= K <= 4
    vmax = 3 if fused else 15
    qclamp = QCLAMP22 if fused else QCLAMP27

    ctx = ExitStack()
    p_w = ctx.enter_context(tc.tile_pool(name="w", bufs=max(2, len(LGROUPS))))
    p_sg = ctx.enter_context(tc.tile_pool(name="sg", bufs=4))
    p_tmp = ctx.enter_context(tc.tile_pool(name="tmp", bufs=4))
    p_map = ctx.enter_context(tc.tile_pool(name="map", bufs=2))
    p_rep = ctx.enter_context(tc.tile_pool(name="rep", bufs=3))
    p_wide = ctx.enter_context(tc.tile_pool(name="wide", bufs=3))
    p_const = ctx.enter_context(tc.tile_pool(name="const", bufs=1))

    def out_dma(d, col0, F, wide):
        rep_b = wide[:].bitcast(u8).unsqueeze(1).broadcast_to([RPC, 4, 4 * F])
        nc.sync.dma_start(out_r[d, :, :, 4 * col0 : 4 * col0 + 4 * F], rep_b)

    if K == 1:
        for _ in range(iters):
            col0 = 0
            for F in CHUNKS:
                for d in range(3):
                    wide = p_wide.tile([RPC, 2 * F], u16, tag=f"wd{d}")
                    nc.vector.memset(wide[:], _b257(pal, surv[0], d))
                    out_dma(d, col0, F, wide)
                col0 += F
        ctx.close()
        return

    # persistent fallback tiles seeding the MAP2 select chains (a [P,1]
    # broadcast Src1 fails on HW; a full 2-D tensor works).  Canvas-width
    # u16, so the memsets are cheap enough to run up front.
    fbw = []
    for d in range(3):
        t = p_const.tile([RPC, max(CHUNKS)], u16, tag=f"fbw{d}")
        nc.vector.memset(t[:], _b257(pal, surv[-1], d))
        fbw.append(t)

    # chunk index -> (load group index, column offset inside the group)
    c2g = []
    goff = []
    gspan = []  # (col0, Fg) per group
    ci = 0
    col0 = 0
    for gi, ng in enumerate(LGROUPS):
        Fg = sum(CHUNKS[ci : ci + ng])
        gspan.append((col0, Fg))
        off = 0
        for F in CHUNKS[ci : ci + ng]:
            c2g.append(gi)
            goff.append(off)
            off += F
            ci += 1
        col0 += Fg

    v2c = {vmax - j: c for j, c in enumerate(surv)}

    for _ in range(iters):
        # all input loads up front: no data deps, the Pool engine paces
        # descriptor generation.  Group 0 goes via the SP HWDGE ring (no
        # Pool startup memsets, faster generation) to cut the critical
        # path to the first output chunk.
        wts = []
        for gi, (gc0, Fg) in enumerate(gspan):
            wt = p_w.tile([RPC, 3 * Fg], f32, tag=f"w{gi}")
            eng = nc.sync if gi == 0 else nc.gpsimd
            eng.dma_start(
                wt[:].rearrange("p (c f) -> p c f", c=3),
                w_ap[:, :, gc0 : gc0 + Fg].rearrange("c p f -> p c f"),
            )
            wts.append(wt)

        def emit_sig(i):
            F = CHUNKS[i]
            wt = wts[c2g[i]]
            Fg = gspan[c2g[i]][1]
            off = goff[i]
            sgt = p_sg.tile([RPC, 3 * F], f32, tag="sg")
            wt_v = wt[:].rearrange("p (c f) -> p c f", c=3)
            nc.scalar.activation(
                sgt[:].rearrange("p (c f) -> p c f", c=3),
                wt_v[:, :, off : off + F], Act.Sigmoid,
            )
            return sgt

        sg_next = emit_sig(0)
        col0 = 0
        for i, F in enumerate(CHUNKS):
            sgt = sg_next
            sg = [sgt[:, d * F : (d + 1) * F] for d in range(3)]

            # --- scores + packed tournament ------------------------------
            pk = None
            for j, c in enumerate(surv):
                u = p_tmp.tile([RPC, F], f32, tag="u")
                nc.vector._custom_dve(
                    SQD2, out=u[:], in0=sg[0], in1=sg[1],
                    s0=float(pal[c, 0]), s1=float(pal[c, 1]),
                )
                sq_ = p_tmp.tile([RPC, F], i32, tag="sq")
                nc.vector._custom_dve(
                    SQDA, out=sq_[:], in0=sg[2], in1=u[:],
                    s0=float(pal[c, 2]), s1=qscale, imm2=qclamp,
                )
                if fused:
                    nk = p_tmp.tile([RPC, F], i32, tag=f"pk{j % 2}")
                    nc.vector._custom_dve(
                        PKMX, out=nk[:], in0=sq_[:],
                        # j == 0: max(s_q*4+3, s_q) == s_q*4+3 seeds it
                        in1=(pk[:] if pk is not None else sq_[:]),
                        s0=float(vmax - j), s1=float(vmax + 1),
                    )
                    pk = nk
                elif j == 0:
                    pk = p_w.tile([RPC, F], i32, tag="packed")
                    nc.vector.tensor_scalar(
                        pk[:], sq_[:], 4, vmax - j,
                        Alu.arith_shift_left, Alu.bitwise_or,
                    )
                else:
                    cand = p_tmp.tile([RPC, F], i32, tag="cand")
                    nc.vector.tensor_scalar(
                        cand[:], sq_[:], 4, vmax - j,
                        Alu.arith_shift_left, Alu.bitwise_or,
                    )
                    # positive IEEE f32 order == int32 order
                    nc.vector.tensor_max(
                        pk[:].bitcast(f32), pk[:].bitcast(f32),
                        cand[:].bitcast(f32),
                    )

            # v = pk & vmax (= vmax - j), cast f32 by the output converter
            idx = p_w.tile([RPC, F], f32, tag="idx")
            nc.vector.tensor_scalar(idx[:], pk[:], vmax, None, Alu.bitwise_and)

            # --- palette map at canvas width, u16 = byte*257 -------------
            rep16 = []
            for d in range(3):
                r16 = p_rep.tile([RPC, F], u16, tag=f"rep{d}")
                if K <= 3:
                    nc.vector._custom_dve(
                        MAP2, out=r16[:], in0=idx[:], in1=fbw[d][:, :F],
                        s0=_b257(pal, v2c[vmax - 1], d) if vmax - 1 in v2c
                        else _b257(pal, surv[0], d),
                        s1=float(vmax - 1),
                        imm2=_b257(pal, v2c[vmax], d),
                    )
                else:
                    vlo = vmax + 1 - K - (K % 2)
                    cur = fbw[d][:, :F]
                    for v in range(vlo, vmax + 1, 2):
                        last = v + 2 > vmax
                        nxt = r16 if last else p_map.tile(
                            [RPC, F], f32, tag=f"m{d}"
                        )
                        nc.vector._custom_dve(
                            MAP2, out=nxt[:], in0=idx[:], in1=cur,
                            s0=_b257(pal, v2c.get(v, surv[-1]), d),
                            s1=float(v),
                            imm2=_b257(pal, v2c.get(v + 1, surv[-1]), d),
                        )
                        cur = nxt[:]
                rep16.append(r16)

            # next chunk's sigmoid goes on the ACT queue BEFORE this
            # chunk's pair-copies so the DVE never waits on it
            if i + 1 < n:
                sg_next = emit_sig(i + 1)

            # --- ACT pair-copy (2nd 2x) + row-replicating store ----------
            for d in range(3):
                wide = p_wide.tile([RPC, 2 * F], u16, tag=f"wd{d}")
                nc.scalar.copy(
                    wide[:],
                    rep16[d][:].unsqueeze(2).broadcast_to([RPC, F, 2]),
                )
                out_dma(d, col0, F, wide)
            col0 += F

    ctx.close()


def build_module(weight, pal):
    """Build + compile the single-core Bass program (palette baked in)."""
    surv, s_ub = _prune_palette(weight, pal)
    K = len(surv)
    if 2 <= K <= 4:
        qscale = float(2.0 ** min(22, int(math.floor(math.log2(QCLAMP22 / s_ub)))))
    else:
        qscale = float(2.0 ** min(30, int(math.floor(math.log2(QCLAMP27 / s_ub)))))
    iters = int(os.environ.get("CBD_ITERS", "1"))
    key = (pal.astype(np.float32).tobytes(), tuple(surv), qscale, iters,
           CHUNKS, LGROUPS)
    if key in _MODULE_CACHE:
        return _MODULE_CACHE[key]

    import concourse.bacc as bacc
    import concourse.mybir as mybir
    import concourse.tile as tile

    nc = bacc.Bacc("TRN2", target_bir_lowering=False, debug=False)
    w_in = nc.dram_tensor("w", [3, RPC, CW], mybir.dt.float32, kind="ExternalInput")
    out = nc.dram_tensor(
        "out", [3, ORPC, OW], mybir.dt.uint8, kind="ExternalOutput"
    )
    with tile.TileContext(nc) as tc:
        _body(tc, nc, out, w_in, pal, surv, qscale, iters=iters)
    nc.compile()
    _MODULE_CACHE[key] = nc
    return nc


def decode_out(a):
    """u8 device output -> f32 colors."""
    return np.asarray(a).astype(np.float32) * (1.0 / 255.0)


def kernel(weight, palette):
    """Full inputs in, full output out. Shards rows across 8 NeuronCores."""
    from concourse.bass_utils import run_bass_kernel_spmd

    weight = np.ascontiguousarray(weight, dtype=np.float32)
    pal = np.ascontiguousarray(palette, dtype=np.float32)
    assert weight.shape == (3, CH, CW) and pal.shape == (NCOLORS, 3)

    nc = build_module(weight, pal)

    in_maps = [
        {"w": np.ascontiguousarray(weight[:, m * RPC : (m + 1) * RPC, :])}
        for m in range(NCORES)
    ]
    trace = bool(int(os.environ.get("CBD_TRACE", "0")))
    res = run_bass_kernel_spmd(
        nc, in_maps, core_ids=list(range(NCORES)), trace=trace
    )
    kernel.last_results = res

    full = np.empty((3, OH, OW), dtype=np.float32)
    for m in range(NCORES):
        full[:, m * ORPC : (m + 1) * ORPC, :] = decode_out(res.results[m]["out"])
    return full
